# revision 9
# baseline (speedup 1.0000x reference)
"""CosineSimAttention Trainium2 kernel.

Math (per batch b):
  q_n = q / ||q||row ; k_n = k / ||k||row
  attn = where(mask, 0, q_n @ k_n.T + 1); attn /= rowsum(attn); out = attn @ v

Key identity used: (q_n . k_n + 1) = (q . k_n + ||q||) / ||q||, and the 1/||q||
row factor cancels in the rowsum normalization.  So the device computes
  s65[q,k] = q . k_n + ||q||        (one matmul with K=65: extra row ||q|| x 1s)
  masked   = s65 * keep             (keep = 1-mask, fused with rowsum on DVE)
  attn     = masked * (1/rowsum)    (graded output)
  out      = (masked @ v) * (1/rowsum)   (via PE transposes of masked)

Sharding: batch dim (32) split 4-per-core across 8 NeuronCores, no collectives.
Host pre-marshals q->qT(+norm row), k->k_nT(+ones row), v->bf16, mask->keep u8.
"""

import os
from contextlib import ExitStack

import ml_dtypes
import numpy as np

import concourse.bass as bass
from concourse import bacc
import concourse.mybir as mybir
import concourse.tile as tile
from concourse.masks import make_identity

F32 = mybir.dt.float32
F32R = mybir.dt.float32r
BF16 = mybir.dt.bfloat16
U8 = mybir.dt.uint8

B, L, D = 32, 2048, 64
N_CORES = 8
B_LOC = B // N_CORES


def build_bass(b_loc=B_LOC, l=L, d=D, keep_dtype=U8):
    """Build the SPMD Bass program for one core (b_loc batches of (l, d))."""
    nc = bacc.Bacc(trn_type="TRN2")
    n_qt = l // 128          # q tiles per batch
    n_kb = l // 512          # 512-wide k blocks
    n_kc = l // 128          # 128-wide k chunks

    qt_d = nc.dram_tensor("qt", (b_loc, d + 1, l), F32R, kind="ExternalInput")
    kt_d = nc.dram_tensor("kt", (b_loc, d + 1, l), F32R, kind="ExternalInput")
    v_d = nc.dram_tensor("v", (b_loc, l, d + 1), BF16, kind="ExternalInput")
    keep_d = nc.dram_tensor("keep", (b_loc, l, l), keep_dtype, kind="ExternalInput")
    attn_d = nc.dram_tensor("attn", (b_loc, l, l), F32, kind="ExternalOutput")
    out_d = nc.dram_tensor("out", (b_loc, l, d), F32, kind="ExternalOutput")

    with tile.TileContext(nc) as tc, ExitStack() as ctx:
        consts = ctx.enter_context(tc.tile_pool(name="consts", bufs=1))
        qkp = ctx.enter_context(tc.tile_pool(name="qkp", bufs=2))
        vp = ctx.enter_context(tc.tile_pool(name="vp", bufs=2))
        keepp = ctx.enter_context(tc.tile_pool(name="keepp", bufs=3))
        maskedp = ctx.enter_context(tc.tile_pool(name="maskedp", bufs=3))
        mtp = ctx.enter_context(tc.tile_pool(name="mtp", bufs=2))
        attnp = ctx.enter_context(tc.tile_pool(name="attnp", bufs=3))
        outp = ctx.enter_context(tc.tile_pool(name="outp", bufs=2))
        smallp = ctx.enter_context(tc.tile_pool(name="smallp", bufs=8))
        ps_s = ctx.enter_context(tc.tile_pool(name="ps_s", bufs=2, space="PSUM"))
        ps_t = ctx.enter_context(tc.tile_pool(name="ps_t", bufs=2, space="PSUM"))
        ps_o = ctx.enter_context(tc.tile_pool(name="ps_o", bufs=2, space="PSUM"))

        ident = consts.tile([128, 128], BF16)
        make_identity(nc, ident)

        for b in range(b_loc):
            qt = qkp.tile([d + 1, l], F32R, tag="qt")
            nc.sync.dma_start(out=qt, in_=qt_d[b])
            kt = qkp.tile([d + 1, l], F32R, tag="kt")
            nc.sync.dma_start(out=kt, in_=kt_d[b])
            vt = vp.tile([128, n_kc, d + 1], BF16, tag="vt")
            nc.sync.dma_start(
                out=vt, in_=v_d[b].rearrange("(c p) d -> p c d", p=128)
            )

            for t in range(n_qt):
                keep_sb = keepp.tile([128, l], keep_dtype, tag="keep")
                nc.sync.dma_start(
                    out=keep_sb, in_=keep_d[b, 128 * t : 128 * (t + 1), :]
                )

                masked = maskedp.tile([128, l], BF16, tag="masked")
                for h in range(2):  # halves of the k dimension
                    hw = l // 2
                    s65 = ps_s.tile([128, hw], F32, tag="s65")
                    for j in range(hw // 512 if hw >= 512 else 1):
                        n = min(512, hw)
                        nc.tensor.matmul(
                            s65[:, j * 512 : j * 512 + n],
                            lhsT=qt[:, 128 * t : 128 * (t + 1)],
                            rhs=kt[:, h * hw + j * 512 : h * hw + j * 512 + n],
                            start=True,
                            stop=True,
                        )
                    nc.vector.tensor_tensor(
                        out=masked[:, h * hw : (h + 1) * hw],
                        in0=s65,
                        in1=keep_sb[:, h * hw : (h + 1) * hw],
                        op=mybir.AluOpType.mult,
                    )
                # masked.T tiles for the attn @ v matmul
                mt = mtp.tile([128, l], BF16, tag="mt")
                for g in range(n_kc // 4):
                    tp = ps_t.tile([128, 512], BF16, tag="tp")
                    for j in range(4):
                        c = 4 * g + j
                        nc.tensor.transpose(
                            tp[:, 128 * j : 128 * (j + 1)],
                            masked[:, 128 * c : 128 * (c + 1)],
                            ident,
                        )
                    nc.scalar.copy(mt[:, 512 * g : 512 * (g + 1)], tp)

                po = ps_o.tile([128, d + 1], F32, tag="po")
                for c in range(n_kc):
                    nc.tensor.matmul(
                        po,
                        lhsT=mt[:, 128 * c : 128 * (c + 1)],
                        rhs=vt[:, c, :],
                        start=(c == 0),
                        stop=(c == n_kc - 1),
                    )
                rs_c = smallp.tile([128, 1], F32, tag="rs_c")
                nc.vector.tensor_scalar_max(rs_c, po[:, d : d + 1], 1e-12)
                recip = smallp.tile([128, 1], F32, tag="recip")
                nc.vector.reciprocal(recip, rs_c)

                attn_sb = attnp.tile([128, l], F32, tag="attn")
                nc.scalar.activation(
                    out=attn_sb,
                    in_=masked,
                    func=mybir.ActivationFunctionType.Copy,
                    scale=recip,
                )
                nc.sync.dma_start(
                    out=attn_d[b, 128 * t : 128 * (t + 1), :], in_=attn_sb
                )
                o_sb = outp.tile([128, d], F32, tag="o")
                nc.scalar.activation(
                    out=o_sb,
                    in_=po[:, :d],
                    func=mybir.ActivationFunctionType.Copy,
                    scale=recip,
                )
                nc.sync.dma_start(
                    out=out_d[b, 128 * t : 128 * (t + 1), :], in_=o_sb
                )

    nc.finalize()
    return nc


def _marshal(q, k, v, mask):
    q = np.ascontiguousarray(np.asarray(q, np.float32))
    k = np.ascontiguousarray(np.asarray(k, np.float32))
    v = np.ascontiguousarray(np.asarray(v, np.float32))
    mask = np.asarray(mask)
    qn = np.linalg.norm(q, axis=2)  # (B, L)
    knorm = np.maximum(np.linalg.norm(k, axis=2, keepdims=True), 1e-12)
    kn = k / knorm
    b, l, d = q.shape
    qt65 = np.empty((b, d + 1, l), np.float32)
    qt65[:, :d] = q.transpose(0, 2, 1)
    qt65[:, d] = qn
    kt65 = np.empty((b, d + 1, l), np.float32)
    kt65[:, :d] = kn.transpose(0, 2, 1)
    kt65[:, d] = 1.0
    vb = np.ones((b, l, d + 1), ml_dtypes.bfloat16)
    vb[:, :, :d] = v.astype(ml_dtypes.bfloat16)
    keep = np.ascontiguousarray((~mask.astype(bool)).astype(np.uint8))
    return qt65, kt65, vb, keep


LAST_EXEC_NS = None


def _run_pjrt(nc, concat_ins, n_cores, n_timing=0):
    """Replicates bass2jax.run_bass_via_pjrt's multi-core path, without output
    donation so the jitted executable can be re-invoked on device-resident
    inputs for steady-state timing (the axon NTFF profile hook is unavailable
    in this container)."""
    import time

    import jax
    import concourse.mybir as mybir_
    from concourse import bass2jax
    from jax.experimental.shard_map import shard_map
    from jax.sharding import Mesh, PartitionSpec

    bass2jax.install_neuronx_cc_hook()
    partition_name = nc.partition_id_tensor.name if nc.partition_id_tensor else None
    in_names, out_names, out_avals = [], [], []
    for alloc in nc.m.functions[0].allocations:
        if not isinstance(alloc, mybir_.MemoryLocationSet):
            continue
        name = alloc.memorylocations[0].name
        if alloc.kind == "ExternalInput":
            if name != partition_name:
                in_names.append(name)
        elif alloc.kind == "ExternalOutput":
            out_names.append(name)
            out_avals.append(
                jax.core.ShapedArray(tuple(alloc.tensor_shape), mybir_.dt.np(alloc.dtype))
            )
    n_params = len(in_names)
    all_in_names = in_names + out_names + ([partition_name] if partition_name else [])

    def _body(*args):
        operands = list(args)
        if partition_name is not None:
            operands.append(bass2jax.partition_id_tensor())
        return tuple(
            bass2jax._bass_exec_p.bind(
                *operands,
                out_avals=tuple(out_avals),
                in_names=tuple(all_in_names),
                out_names=tuple(out_names),
                lowering_input_output_aliases=(),
                sim_require_finite=True,
                sim_require_nnan=True,
                nc=nc,
            )
        )

    devices = jax.devices()[:n_cores]
    mesh = Mesh(np.asarray(devices), ("core",))
    nin = n_params + len(out_names)
    sharded = jax.jit(
        shard_map(
            _body,
            mesh=mesh,
            in_specs=(PartitionSpec("core"),) * nin,
            out_specs=(PartitionSpec("core"),) * len(out_names),
            check_rep=False,
        ),
        keep_unused=True,
    )
    sh = jax.sharding.NamedSharding(mesh, PartitionSpec("core"))
    args = [jax.device_put(concat_ins[name], sh) for name in in_names]
    args += [
        jax.device_put(
            np.zeros((n_cores * a.shape[0], *a.shape[1:]), a.dtype), sh
        )
        for a in out_avals
    ]
    outs = sharded(*args)
    jax.block_until_ready(outs)
    exec_ns = None
    if n_timing > 0:
        times = []
        for _ in range(n_timing):
            t0 = time.perf_counter()
            o = sharded(*args)
            jax.block_until_ready(o)
            times.append(time.perf_counter() - t0)
        exec_ns = int(min(times) * 1e9)
    return {name: np.asarray(outs[i]) for i, name in enumerate(out_names)}, exec_ns


def kernel(q, k, v, mask):
    global LAST_EXEC_NS
    qt65, kt65, vb, keep = _marshal(q, k, v, mask)
    nc = build_bass()
    concat_ins = {"qt": qt65, "kt": kt65, "v": vb, "keep": keep}
    n_timing = int(os.environ.get("KBENCH_TIME", "0"))
    outs, exec_ns = _run_pjrt(nc, concat_ins, N_CORES, n_timing=n_timing)
    LAST_EXEC_NS = exec_ns
    return outs["out"], outs["attn"]
